# revision 7
# baseline (speedup 1.0000x reference)
"""Multi-head causal attention (B=2, T=2048, C=1024, H=16) on 8 TRN2 cores.

Sharding: core c -> batch b = c//4, head-group hg = c%4 (4 heads, 256 dims).
Data parallel on B, tensor parallel on H. Host pre-transposes x and weight
slices; device computes in f32r (full-rate fp32 container) with fp32 PSUM
accumulation; host sums tensor-parallel partials.

Device phases per core:
  1. qT,kT [256,2048] (transposed), v natural [2048,256] (+bias via ones-row MM)
  2. nat-pass per t-tile: scores[t,s] -> +mask -> exp(accum_out=rowsums)
     -> 1/Z columns -> avg_part[t,s] += diag(1/Z_h) @ E_h (PE accumulates heads)
  3. 1/Z rows via PE transpose of the spread 1/Z columns
  4. T-pass per head pair: scoresT[s,t] -> +mask -> exp -> E_T;
     attnT[128,t] += v_pair.T @ E_T (two heads share PSUM via col groups);
     normalize with broadcast 1/Z rows
  5. out_part = attnT_n.T @ Wo.T slice
"""
import math
import sys

sys.path.insert(0, '/opt/trn_rl_repo')

import numpy as np

import concourse.bass as bass
import concourse.tile as tile
from concourse import bacc, mybir
from concourse.bass import ts, ds
from concourse.bass_utils import run_bass_kernel_spmd

F32, F32R = mybir.dt.float32, mybir.dt.float32r
AF = mybir.ActivationFunctionType
ADD, MULT = mybir.AluOpType.add, mybir.AluOpType.mult

B, T, C, H = 2, 2048, 1024, 16
DH = C // H                      # 64
HG = 4                           # heads per core
HD = HG * DH                     # 256
NKT = C // 128                   # 8
NTT = T // 128                   # 16
SCALE = 1.0 / math.sqrt(DH)
NEG = -1.0e30

_CACHE = {}


def _chunks(sb, se):
    """512-aligned chunks covering [sb, se)."""
    out = []
    cb = sb
    while cb < se:
        cw = min(512 * (cb // 512 + 1), se) - cb
        out.append((cb, cw))
        cb += cw
    return out


def _build():
    nc = bacc.Bacc("TRN2", target_bir_lowering=False, debug=False, num_devices=8)
    xt = nc.dram_tensor("xt", [C, T], F32, kind="ExternalInput")
    wqt = nc.dram_tensor("wqt", [C, HD], F32, kind="ExternalInput")
    wkt = nc.dram_tensor("wkt", [C, HD], F32, kind="ExternalInput")
    wvt = nc.dram_tensor("wvt", [C, HD], F32, kind="ExternalInput")
    wot = nc.dram_tensor("wot", [HD, C], F32, kind="ExternalInput")
    bqm = nc.dram_tensor("bqm", [128, 2], F32, kind="ExternalInput")
    bkm = nc.dram_tensor("bkm", [128, 2], F32, kind="ExternalInput")
    bvr = nc.dram_tensor("bvr", [1, HD], F32, kind="ExternalInput")
    mneg_t = nc.dram_tensor("mneg_t", [128, 128], F32, kind="ExternalInput")
    mneg_n = nc.dram_tensor("mneg_n", [128, 128], F32, kind="ExternalInput")
    ident = nc.dram_tensor("ident", [128, 128], F32, kind="ExternalInput")
    out_part = nc.dram_tensor("out_part", [T, C], F32, kind="ExternalOutput")
    avg_part = nc.dram_tensor("avg_part", [T, T], F32, kind="ExternalOutput")

    with tile.TileContext(nc) as tc:
        with tc.tile_pool(name="persist", bufs=1) as pp:
            # ---------- persistent tiles ----------
            qt_s = pp.tile([128, 2, T], F32R, tag="qt")
            kt_s = pp.tile([128, 2, T], F32R, tag="kt")
            vaug = pp.tile([128, NTT, HD], F32R, tag="vaug")
            attnT_n = pp.tile([128, 2, T], F32R, tag="attnT_n")
            wot_s = pp.tile([128, 2, C], F32R, tag="wot")
            nc.gpsimd.dma_start(
                wot_s[:], wot.rearrange("(kt p) n -> p kt n", p=128).bitcast(F32R))
            bq_s = pp.tile([128, 2], F32, tag="bq")
            bk_s = pp.tile([128, 2], F32, tag="bk")
            nc.gpsimd.dma_start(bq_s[:], bqm[:])
            nc.gpsimd.dma_start(bk_s[:], bkm[:])
            bv_row = pp.tile([1, HD], F32R, tag="bv_row")
            nc.gpsimd.dma_start(bv_row[:], bvr[:].bitcast(F32R))
            ones_row = pp.tile([1, 128], F32R, tag="ones_row")
            nc.vector.memset(ones_row[:].bitcast(F32), 1.0)
            mt_s = pp.tile([128, 128], F32, tag="mt_s")
            mn_s = pp.tile([128, 128], F32, tag="mn_s")
            id_s = pp.tile([128, 128], F32, tag="id_s")
            nc.gpsimd.dma_start(mt_s[:], mneg_t[:])
            nc.gpsimd.dma_start(mn_s[:], mneg_n[:])
            nc.gpsimd.dma_start(id_s[:], ident[:])
            # 1/Z row form per head (partition 0 only)
            rzr = [pp.tile([1, T], F32, tag=f"rzr{h}", name=f"rzr{h}")
                   for h in range(HG)]

            # ---------- phase 1: projections ----------
            with (
                tc.tile_pool(name="ph1", bufs=1) as p1,
                tc.tile_pool(name="ph1ps", bufs=1, space="PSUM") as p1ps,
            ):
                xt_s = p1.tile([128, NKT, T], F32R, tag="xt")
                wq_s = p1.tile([128, NKT, HD], F32R, tag="wq")
                wk_s = p1.tile([128, NKT, HD], F32R, tag="wk")
                wv_s = p1.tile([128, NKT, HD], F32R, tag="wv")
                nc.gpsimd.dma_start(
                    wq_s[:], wqt.rearrange("(kt p) n -> p kt n", p=128).bitcast(F32R))
                for k in range(NKT):
                    nc.gpsimd.dma_start(
                        xt_s[:, k, :], xt[ts(k, 128), :].bitcast(F32R))
                for w_t, dram in [(wk_s, wkt), (wv_s, wvt)]:
                    for k in range(NKT):
                        nc.gpsimd.dma_start(
                            w_t[:, k, :], dram[ts(k, 128), :].bitcast(F32R))

                for dst, w_t, b_t in [(qt_s, wq_s, bq_s), (kt_s, wk_s, bk_s)]:
                    for m in range(2):
                        for tch in range(T // 512):
                            ps = p1ps.tile([128, 512], F32, tag="pj", bufs=2,
                                           name=f"pj{m}_{tch}")
                            for k in range(NKT):
                                nc.tensor.matmul(
                                    ps[:], w_t[:, k, ts(m, 128)],
                                    xt_s[:, k, ts(tch, 512)],
                                    start=(k == 0), stop=(k == NKT - 1))
                            dslice = dst[:, m, ts(tch, 512)]
                            nc.scalar.activation(dslice, ps[:], AF.Copy)
                            nc.vector.tensor_scalar_add(dslice, dslice,
                                                        b_t[:, m:m + 1])
                for j in range(NTT):
                    ps = p1ps.tile([128, HD], F32, tag="pj", bufs=2, name=f"pv{j}")
                    for k in range(NKT):
                        nc.tensor.matmul(ps[:], xt_s[:, k, ts(j, 128)],
                                         wv_s[:, k, :], start=(k == 0), stop=False)
                    nc.tensor.matmul(ps[:], ones_row[:], bv_row[:],
                                     start=False, stop=True)
                    nc.scalar.activation(vaug[:, j, :], ps[:], AF.Copy)

            with (
                tc.tile_pool(name="ph2", bufs=1) as p2,
                tc.tile_pool(name="ph2ps", bufs=1, space="PSUM") as ps2,
            ):
                # ---------- phase 2: nat-pass (avg + rowsums) ----------
                for tt in range(NTT):
                    width = 128 * (tt + 1)
                    nseg = (width + 1023) // 1024
                    racc = p2.tile([128, HG, 2], F32, tag="racc", bufs=2,
                                   name=f"racc{tt}")
                    e_nat = []
                    for h in range(HG):
                        mt, hb = h // 2, 64 * (h % 2)
                        en = p2.tile([128, T], F32R, tag=f"en{h}", bufs=(2 if h < 2 else 1), name=f"en{tt}_{h}")
                        e_nat.append(en)
                        for si in range(nseg):
                            sb, se = 1024 * si, min(1024 * si + 1024, width)
                            sn = ps2.tile([128, 1024], F32, tag="span", bufs=2,
                                          name=f"sn{tt}_{h}_{si}")
                            for (cb, cw) in _chunks(sb, se):
                                nc.tensor.matmul(
                                    sn[:, ds(cb - sb, cw)],
                                    qt_s[hb:hb + DH, mt, ts(tt, 128)],
                                    kt_s[hb:hb + DH, mt, ds(cb, cw)],
                                    start=True, stop=True)
                            if se == width:  # diagonal block lives here
                                nc.vector.tensor_add(
                                    sn[:, ds(width - 128 - sb, 128)],
                                    sn[:, ds(width - 128 - sb, 128)], mn_s[:])
                            nc.scalar.activation(en[:, ds(sb, se - sb)],
                                                 sn[:, 0:se - sb], AF.Exp,
                                                 scale=SCALE,
                                                 accum_out=racc[:, h, si:si + 1])
                        if nseg < 2:
                            nc.vector.memset(racc[:, h, 1:2], 0.0)
                    zs = p2.tile([128, HG], F32, tag="zs", bufs=2, name=f"zs{tt}")
                    nc.vector.tensor_reduce(zs[:], racc[:],
                                            axis=mybir.AxisListType.X, op=ADD)
                    rz4 = p2.tile([128, HG], F32, tag="rz4", bufs=2, name=f"rz4{tt}")
                    nc.vector.reciprocal(rz4[:], zs[:])
                    rzcs = p2.tile([128, 128], F32, tag="rzcs", bufs=2,
                                   name=f"rzcs{tt}")
                    nc.vector.tensor_copy(
                        rzcs[:].rearrange("p (h q) -> p h q", q=32)[:, :, 0:1],
                        rz4[:].rearrange("p (h q) -> p h q", q=1))
                    # 1/Z row form for the T-pass (PE transpose + row copies)
                    tpz = ps2.tile([128, 128], F32, tag="span", bufs=2,
                                   name=f"tpz{tt}")
                    nc.tensor.transpose(tpz[:], rzcs[:], id_s[:])
                    for h in range(HG):
                        nc.vector.tensor_copy(rzr[h][:, ts(tt, 128)],
                                              tpz[32 * h:32 * h + 1, :])
                    # avg accumulation over heads via diagonal matmuls
                    av = ps2.tile([128, T], F32, tag="acc", name=f"av{tt}")
                    for h in range(HG):
                        dg = p2.tile([128, 128], F32R, tag="dg", bufs=2,
                                     name=f"dg{tt}_{h}")
                        nc.vector.tensor_scalar(dg[:], id_s[:],
                                                rzcs[:, 32 * h:32 * h + 1],
                                                None, op0=MULT)
                        for (cb, cw) in _chunks(0, width):
                            nc.tensor.matmul(av[:, ds(cb, cw)], dg[:],
                                             e_nat[h][:, ds(cb, cw)],
                                             start=(h == 0), stop=(h == HG - 1))
                    avs = p2.tile([128, T], F32, tag="avs", bufs=1, name=f"avs{tt}")
                    nc.vector.tensor_scalar_mul(avs[:, 0:width], av[:, 0:width],
                                                1.0 / H)
                    nc.gpsimd.dma_start(avg_part[ts(tt, 128), 0:width],
                                        avs[:, 0:width])

                # ---------- phase 4+5: T-pass (t-half outer) + out-proj ----------
                for tc2 in range(2):
                    hbase, hend = 1024 * tc2, 1024 * (tc2 + 1)
                    for mp in range(2):
                        # both heads' unnormalized attnT: [64, head, 1024]
                        a2 = ps2.tile([64, 2, 1024], F32, tag="acc",
                                      name=f"a2_{mp}_{tc2}")
                        for j in range(8 * (tc2 + 1)):
                            sb = max(128 * j, hbase)
                            sc = {}
                            et = {}
                            for h2 in range(2):
                                sc[h2] = ps2.tile([128, 1024], F32, tag="span",
                                                  bufs=2,
                                                  name=f"sc{mp}_{tc2}_{j}_{h2}")
                                et[h2] = p2.tile([128, 1024], F32R, tag="et",
                                                 bufs=3,
                                                 name=f"et{mp}_{tc2}_{j}_{h2}")
                            # interleave heads chunk-by-chunk (disjoint PE rows)
                            for (cb, cw) in _chunks(sb, hend):
                                for h2 in range(2):
                                    hb = 64 * h2
                                    nc.tensor.matmul(
                                        sc[h2][:, ds(cb - hbase, cw)],
                                        kt_s[hb:hb + DH, mp, ts(j, 128)],
                                        qt_s[hb:hb + DH, mp, ds(cb, cw)],
                                        start=True, stop=True)
                            for h2 in range(2):
                                if sb == 128 * j:  # diagonal block in this half
                                    nc.vector.tensor_add(
                                        sc[h2][:, ds(sb - hbase, 128)],
                                        sc[h2][:, ds(sb - hbase, 128)], mt_s[:])
                                nc.scalar.activation(
                                    et[h2][:, ds(sb - hbase, hend - sb)],
                                    sc[h2][:, ds(sb - hbase, hend - sb)],
                                    AF.Exp, scale=SCALE)
                            for (cb, cw) in _chunks(sb, hend):
                                for h2 in range(2):
                                    nc.tensor.matmul(
                                        a2[:, h2, ds(cb - hbase, cw)],
                                        vaug[:, j, ts(2 * mp + h2, DH)],
                                        et[h2][:, ds(cb - hbase, cw)],
                                        start=(j == 0),
                                        stop=(j % 4 == 3 and cb // 512 == j // 4))
                        for h2 in range(2):
                            hb = 64 * h2
                            rb = p2.tile([DH, 1024], F32, tag="rb", bufs=1,
                                         name=f"rb{mp}_{tc2}_{h2}")
                            nc.gpsimd.partition_broadcast(
                                rb[:], rzr[2 * mp + h2][:, ds(hbase, 1024)])
                            nc.vector.tensor_mul(
                                attnT_n[hb:hb + DH, mp, ds(hbase, 1024)],
                                a2[:, h2, :], rb[:])
                    # out-projection for this t-half (overlaps next half's T-pass)
                    for tt in range(8 * tc2, 8 * tc2 + 8):
                        po = ps2.tile([128, C], F32, tag="span", bufs=2,
                                      name=f"po{tt}")
                        for nch in range(2):
                            for m in range(2):
                                nc.tensor.matmul(po[:, ts(nch, 512)],
                                                 attnT_n[:, m, ts(tt, 128)],
                                                 wot_s[:, m, ts(nch, 512)],
                                                 start=(m == 0), stop=(m == 1))
                        osb = p2.tile([128, C], F32, tag="osb", bufs=2,
                                      name=f"osb{tt}")
                        nc.vector.tensor_copy(osb[:], po[:])
                        nc.gpsimd.dma_start(out_part[ts(tt, 128), :], osb[:])

    nc.compile()
    return nc


def kernel(x, Wq, bq, Wk, bk, Wv, bv, Wo, bo):
    x = np.asarray(x, dtype=np.float32)
    Wq, Wk, Wv, Wo = (np.asarray(a, dtype=np.float32) for a in (Wq, Wk, Wv, Wo))
    bq, bk, bv, bo = (np.asarray(a, dtype=np.float32) for a in (bq, bk, bv, bo))

    if "nc" not in _CACHE:
        _CACHE["nc"] = _build()
    nc = _CACHE["nc"]

    r = np.arange(128)
    mneg = np.where(r[:, None] <= r[None, :], 0.0, NEG).astype(np.float32)
    ident = np.eye(128, dtype=np.float32)

    in_maps = []
    for core in range(8):
        b, hg = core // 4, core % 4
        sl = slice(hg * HD, (hg + 1) * HD)
        in_maps.append({
            "xt": np.ascontiguousarray(x[b].T),
            "wqt": np.ascontiguousarray(Wq[sl, :].T),
            "wkt": np.ascontiguousarray(Wk[sl, :].T),
            "wvt": np.ascontiguousarray(Wv[sl, :].T),
            "wot": np.ascontiguousarray(Wo[:, sl].T),
            "bqm": np.ascontiguousarray(bq[sl].reshape(2, 128).T),
            "bkm": np.ascontiguousarray(bk[sl].reshape(2, 128).T),
            "bvr": np.ascontiguousarray(bv[sl].reshape(1, HD)),
            "mneg_t": mneg,                       # [s,t] keep s<=t
            "mneg_n": np.ascontiguousarray(mneg.T),  # [t,s] keep s<=t
            "ident": ident,
        })

    res = run_bass_kernel_spmd(nc, in_maps, core_ids=list(range(8)),
                               **_CACHE.get("run_kwargs", {}))
    _CACHE["last_result"] = res

    out = np.zeros((B, T, C), dtype=np.float32)
    avg = np.zeros((B, T, T), dtype=np.float32)
    for core in range(8):
        b = core // 4
        out[b] += res.results[core]["out_part"]
        avg[b] += res.results[core]["avg_part"]
    out += bo
    return out, avg


# revision 8
# speedup vs baseline: 1.0573x; 1.0573x over previous
"""Multi-head causal attention (B=2, T=2048, C=1024, H=16) on 8 TRN2 cores.

Sharding: core c -> batch b = c//4, head-group hg = c%4 (4 heads, 256 dims).
Data parallel on B, tensor parallel on H. Host pre-transposes x and weight
slices; device computes in f32r (full-rate fp32 container) with fp32 PSUM
accumulation; host sums tensor-parallel partials.

Device phases per core:
  1. qT,kT [256,2048] (transposed), v natural [2048,256] (+bias via ones-row MM)
  2. nat-pass per t-tile: scores[t,s] -> +mask -> exp(accum_out=rowsums)
     -> 1/Z columns -> avg_part[t,s] += diag(1/Z_h) @ E_h (PE accumulates heads)
  3. 1/Z rows via PE transpose of the spread 1/Z columns
  4. T-pass per head pair: scoresT[s,t] -> +mask -> exp -> E_T;
     attnT[128,t] += v_pair.T @ E_T (two heads share PSUM via col groups);
     normalize with broadcast 1/Z rows
  5. out_part = attnT_n.T @ Wo.T slice
"""
import math
import sys

sys.path.insert(0, '/opt/trn_rl_repo')

import numpy as np

import concourse.bass as bass
import concourse.tile as tile
from concourse import bacc, mybir
from concourse.bass import ts, ds
from concourse.bass_utils import run_bass_kernel_spmd

F32, F32R = mybir.dt.float32, mybir.dt.float32r
AF = mybir.ActivationFunctionType
ADD, MULT = mybir.AluOpType.add, mybir.AluOpType.mult

B, T, C, H = 2, 2048, 1024, 16
DH = C // H                      # 64
HG = 4                           # heads per core
HD = HG * DH                     # 256
NKT = C // 128                   # 8
NTT = T // 128                   # 16
SCALE = 1.0 / math.sqrt(DH)
NEG = -1.0e30

_CACHE = {}


def _chunks(sb, se):
    """512-aligned chunks covering [sb, se)."""
    out = []
    cb = sb
    while cb < se:
        cw = min(512 * (cb // 512 + 1), se) - cb
        out.append((cb, cw))
        cb += cw
    return out


def _build():
    nc = bacc.Bacc("TRN2", target_bir_lowering=False, debug=False, num_devices=8)
    xt = nc.dram_tensor("xt", [C, T], F32, kind="ExternalInput")
    wqt = nc.dram_tensor("wqt", [C, HD], F32, kind="ExternalInput")
    wkt = nc.dram_tensor("wkt", [C, HD], F32, kind="ExternalInput")
    wvt = nc.dram_tensor("wvt", [C, HD], F32, kind="ExternalInput")
    wot = nc.dram_tensor("wot", [HD, C], F32, kind="ExternalInput")
    bqm = nc.dram_tensor("bqm", [128, 2], F32, kind="ExternalInput")
    bkm = nc.dram_tensor("bkm", [128, 2], F32, kind="ExternalInput")
    bvr = nc.dram_tensor("bvr", [1, HD], F32, kind="ExternalInput")
    mneg_t = nc.dram_tensor("mneg_t", [128, 128], F32, kind="ExternalInput")
    mneg_n = nc.dram_tensor("mneg_n", [128, 128], F32, kind="ExternalInput")
    ident = nc.dram_tensor("ident", [128, 128], F32, kind="ExternalInput")
    out_part = nc.dram_tensor("out_part", [T, C], F32, kind="ExternalOutput")
    avg_part = nc.dram_tensor("avg_part", [T, T], F32, kind="ExternalOutput")

    with tile.TileContext(nc) as tc:
        with tc.tile_pool(name="persist", bufs=1) as pp:
            # ---------- persistent tiles ----------
            qt_s = pp.tile([128, 2, T], F32R, tag="qt")
            kt_s = pp.tile([128, 2, T], F32R, tag="kt")
            vaug = pp.tile([128, NTT, HD], F32R, tag="vaug")
            attnT_n = pp.tile([128, 2, T], F32R, tag="attnT_n")
            wot_s = pp.tile([128, 2, C], F32R, tag="wot")
            nc.gpsimd.dma_start(
                wot_s[:], wot.rearrange("(kt p) n -> p kt n", p=128).bitcast(F32R))
            bq_s = pp.tile([128, 2], F32, tag="bq")
            bk_s = pp.tile([128, 2], F32, tag="bk")
            nc.gpsimd.dma_start(bq_s[:], bqm[:])
            nc.gpsimd.dma_start(bk_s[:], bkm[:])
            bv_row = pp.tile([1, HD], F32R, tag="bv_row")
            nc.gpsimd.dma_start(bv_row[:], bvr[:].bitcast(F32R))
            ones_row = pp.tile([1, 128], F32R, tag="ones_row")
            nc.vector.memset(ones_row[:].bitcast(F32), 1.0)
            mt_s = pp.tile([128, 128], F32, tag="mt_s")
            mn_s = pp.tile([128, 128], F32, tag="mn_s")
            id_s = pp.tile([128, 128], F32, tag="id_s")
            nc.gpsimd.dma_start(mt_s[:], mneg_t[:])
            nc.gpsimd.dma_start(mn_s[:], mneg_n[:])
            nc.gpsimd.dma_start(id_s[:], ident[:])
            # 1/Z row form per head (partition 0 only)
            rzr = [pp.tile([1, T], F32, tag=f"rzr{h}", name=f"rzr{h}")
                   for h in range(HG)]

            # ---------- phase 1: projections ----------
            with (
                tc.tile_pool(name="ph1", bufs=1) as p1,
                tc.tile_pool(name="ph1ps", bufs=1, space="PSUM") as p1ps,
            ):
                xt_s = p1.tile([128, NKT, T], F32R, tag="xt")
                wq_s = p1.tile([128, NKT, HD], F32R, tag="wq")
                wk_s = p1.tile([128, NKT, HD], F32R, tag="wk")
                wv_s = p1.tile([128, NKT, HD], F32R, tag="wv")
                nc.gpsimd.dma_start(
                    wq_s[:], wqt.rearrange("(kt p) n -> p kt n", p=128).bitcast(F32R))
                for k in range(NKT):
                    nc.gpsimd.dma_start(
                        xt_s[:, k, :], xt[ts(k, 128), :].bitcast(F32R))
                for w_t, dram in [(wk_s, wkt), (wv_s, wvt)]:
                    for k in range(NKT):
                        nc.gpsimd.dma_start(
                            w_t[:, k, :], dram[ts(k, 128), :].bitcast(F32R))

                for dst, w_t, b_t in [(qt_s, wq_s, bq_s), (kt_s, wk_s, bk_s)]:
                    for m in range(2):
                        for tch in range(T // 512):
                            ps = p1ps.tile([128, 512], F32, tag="pj", bufs=2,
                                           name=f"pj{m}_{tch}")
                            for k in range(NKT):
                                nc.tensor.matmul(
                                    ps[:], w_t[:, k, ts(m, 128)],
                                    xt_s[:, k, ts(tch, 512)],
                                    start=(k == 0), stop=(k == NKT - 1))
                            dslice = dst[:, m, ts(tch, 512)]
                            nc.scalar.activation(dslice, ps[:], AF.Copy)
                            nc.vector.tensor_scalar_add(dslice, dslice,
                                                        b_t[:, m:m + 1])
                for j in range(NTT):
                    ps = p1ps.tile([128, HD], F32, tag="pj", bufs=2, name=f"pv{j}")
                    for k in range(NKT):
                        nc.tensor.matmul(ps[:], xt_s[:, k, ts(j, 128)],
                                         wv_s[:, k, :], start=(k == 0), stop=False)
                    nc.tensor.matmul(ps[:], ones_row[:], bv_row[:],
                                     start=False, stop=True)
                    nc.scalar.activation(vaug[:, j, :], ps[:], AF.Copy)

            with (
                tc.tile_pool(name="ph2", bufs=1) as p2,
                tc.tile_pool(name="ph2ps", bufs=1, space="PSUM") as ps2,
            ):
                # ---------- phase 2: nat-pass (avg + rowsums) ----------
                for tt in range(NTT):
                    width = 128 * (tt + 1)
                    nseg = (width + 1023) // 1024
                    racc = p2.tile([128, HG, 2], F32, tag="racc", bufs=2,
                                   name=f"racc{tt}")
                    e_nat = []
                    for h in range(HG):
                        mt, hb = h // 2, 64 * (h % 2)
                        en = p2.tile([128, T], F32R, tag=f"en{h}", bufs=(2 if h < 2 else 1), name=f"en{tt}_{h}")
                        e_nat.append(en)
                        for si in range(nseg):
                            sb, se = 1024 * si, min(1024 * si + 1024, width)
                            sn = ps2.tile([128, 1024], F32, tag="span", bufs=2,
                                          name=f"sn{tt}_{h}_{si}")
                            for (cb, cw) in _chunks(sb, se):
                                nc.tensor.matmul(
                                    sn[:, ds(cb - sb, cw)],
                                    qt_s[hb:hb + DH, mt, ts(tt, 128)],
                                    kt_s[hb:hb + DH, mt, ds(cb, cw)],
                                    start=True, stop=True)
                            if se == width:  # diagonal block lives here
                                nc.vector.tensor_add(
                                    sn[:, ds(width - 128 - sb, 128)],
                                    sn[:, ds(width - 128 - sb, 128)], mn_s[:])
                            nc.scalar.activation(en[:, ds(sb, se - sb)],
                                                 sn[:, 0:se - sb], AF.Exp,
                                                 scale=SCALE,
                                                 accum_out=racc[:, h, si:si + 1])
                        if nseg < 2:
                            nc.vector.memset(racc[:, h, 1:2], 0.0)
                    zs = p2.tile([128, HG], F32, tag="zs", bufs=2, name=f"zs{tt}")
                    nc.vector.tensor_reduce(zs[:], racc[:],
                                            axis=mybir.AxisListType.X, op=ADD)
                    rz4 = p2.tile([128, HG], F32, tag="rz4", bufs=2, name=f"rz4{tt}")
                    nc.vector.reciprocal(rz4[:], zs[:])
                    rzcs = p2.tile([128, 128], F32, tag="rzcs", bufs=2,
                                   name=f"rzcs{tt}")
                    nc.vector.tensor_copy(
                        rzcs[:].rearrange("p (h q) -> p h q", q=32)[:, :, 0:1],
                        rz4[:].rearrange("p (h q) -> p h q", q=1))
                    # 1/Z row form for the T-pass (PE transpose + row copies)
                    tpz = ps2.tile([128, 128], F32, tag="span", bufs=2,
                                   name=f"tpz{tt}")
                    nc.tensor.transpose(tpz[:], rzcs[:], id_s[:])
                    for h in range(HG):
                        nc.vector.tensor_copy(rzr[h][:, ts(tt, 128)],
                                              tpz[32 * h:32 * h + 1, :])
                    # avg accumulation over heads via diagonal matmuls
                    av = ps2.tile([128, T], F32, tag="acc", name=f"av{tt}")
                    for h in range(HG):
                        dg = p2.tile([128, 128], F32R, tag="dg", bufs=2,
                                     name=f"dg{tt}_{h}")
                        nc.vector.tensor_scalar(dg[:], id_s[:],
                                                rzcs[:, 32 * h:32 * h + 1],
                                                None, op0=MULT)
                        for (cb, cw) in _chunks(0, width):
                            nc.tensor.matmul(av[:, ds(cb, cw)], dg[:],
                                             e_nat[h][:, ds(cb, cw)],
                                             start=(h == 0), stop=(h == HG - 1))
                    avs = p2.tile([128, T], F32, tag="avs", bufs=1, name=f"avs{tt}")
                    nc.vector.tensor_scalar_mul(avs[:, 0:width], av[:, 0:width],
                                                1.0 / H)
                    nc.gpsimd.dma_start(avg_part[ts(tt, 128), 0:width],
                                        avs[:, 0:width])

                # ---------- phase 4: T-pass per (head pair, t-half) ----------
                for mp in range(2):
                    for tc2 in range(2):
                        hbase, hend = 1024 * tc2, 1024 * (tc2 + 1)
                        a2 = ps2.tile([64, 2, 1024], F32, tag="acc",
                                      name=f"a2_{mp}_{tc2}")
                        for j in range(8 * (tc2 + 1)):
                            sb = max(128 * j, hbase)
                            for h2 in range(2):
                                hb = 64 * h2
                                sc = ps2.tile([128, 1024], F32, tag="span", bufs=2,
                                              name=f"sc{mp}_{tc2}_{j}_{h2}")
                                for (cb, cw) in _chunks(sb, hend):
                                    nc.tensor.matmul(
                                        sc[:, ds(cb - hbase, cw)],
                                        kt_s[hb:hb + DH, mp, ts(j, 128)],
                                        qt_s[hb:hb + DH, mp, ds(cb, cw)],
                                        start=True, stop=True)
                                if sb == 128 * j:  # diagonal block in this half
                                    nc.vector.tensor_add(
                                        sc[:, ds(sb - hbase, 128)],
                                        sc[:, ds(sb - hbase, 128)], mt_s[:])
                                et = p2.tile([128, 1024], F32R, tag="et", bufs=3,
                                             name=f"et{mp}_{tc2}_{j}_{h2}")
                                nc.scalar.activation(et[:, ds(sb - hbase, hend - sb)],
                                                     sc[:, ds(sb - hbase, hend - sb)],
                                                     AF.Exp, scale=SCALE)
                                for (cb, cw) in _chunks(sb, hend):
                                    nc.tensor.matmul(
                                        a2[:, h2, ds(cb - hbase, cw)],
                                        vaug[:, j, ts(2 * mp + h2, DH)],
                                        et[:, ds(cb - hbase, cw)],
                                        start=(j == 0),
                                        stop=(j % 4 == 3 and cb // 512 == j // 4))
                        for h2 in range(2):
                            hb = 64 * h2
                            rb = p2.tile([DH, 1024], F32, tag="rb", bufs=1,
                                         name=f"rb{mp}_{tc2}_{h2}")
                            nc.gpsimd.partition_broadcast(
                                rb[:], rzr[2 * mp + h2][:, ds(hbase, 1024)])
                            nc.vector.tensor_mul(
                                attnT_n[hb:hb + DH, mp, ds(hbase, 1024)],
                                a2[:, h2, :], rb[:])

                # ---------- phase 5: out-projection ----------
                for tt in range(NTT):
                    po = ps2.tile([128, C], F32, tag="span", bufs=2, name=f"po{tt}")
                    for nch in range(2):
                        for m in range(2):
                            nc.tensor.matmul(po[:, ts(nch, 512)],
                                             attnT_n[:, m, ts(tt, 128)],
                                             wot_s[:, m, ts(nch, 512)],
                                             start=(m == 0), stop=(m == 1))
                    osb = p2.tile([128, C], F32, tag="osb", bufs=2, name=f"osb{tt}")
                    if tt % 2 == 0:
                        nc.scalar.activation(osb[:], po[:], AF.Copy)
                    else:
                        nc.vector.tensor_copy(osb[:], po[:])
                    nc.gpsimd.dma_start(out_part[ts(tt, 128), :], osb[:])

    nc.compile()
    return nc


def kernel(x, Wq, bq, Wk, bk, Wv, bv, Wo, bo):
    x = np.asarray(x, dtype=np.float32)
    Wq, Wk, Wv, Wo = (np.asarray(a, dtype=np.float32) for a in (Wq, Wk, Wv, Wo))
    bq, bk, bv, bo = (np.asarray(a, dtype=np.float32) for a in (bq, bk, bv, bo))

    if "nc" not in _CACHE:
        _CACHE["nc"] = _build()
    nc = _CACHE["nc"]

    r = np.arange(128)
    mneg = np.where(r[:, None] <= r[None, :], 0.0, NEG).astype(np.float32)
    ident = np.eye(128, dtype=np.float32)

    in_maps = []
    for core in range(8):
        b, hg = core // 4, core % 4
        sl = slice(hg * HD, (hg + 1) * HD)
        in_maps.append({
            "xt": np.ascontiguousarray(x[b].T),
            "wqt": np.ascontiguousarray(Wq[sl, :].T),
            "wkt": np.ascontiguousarray(Wk[sl, :].T),
            "wvt": np.ascontiguousarray(Wv[sl, :].T),
            "wot": np.ascontiguousarray(Wo[:, sl].T),
            "bqm": np.ascontiguousarray(bq[sl].reshape(2, 128).T),
            "bkm": np.ascontiguousarray(bk[sl].reshape(2, 128).T),
            "bvr": np.ascontiguousarray(bv[sl].reshape(1, HD)),
            "mneg_t": mneg,                       # [s,t] keep s<=t
            "mneg_n": np.ascontiguousarray(mneg.T),  # [t,s] keep s<=t
            "ident": ident,
        })

    res = run_bass_kernel_spmd(nc, in_maps, core_ids=list(range(8)),
                               **_CACHE.get("run_kwargs", {}))
    _CACHE["last_result"] = res

    out = np.zeros((B, T, C), dtype=np.float32)
    avg = np.zeros((B, T, T), dtype=np.float32)
    for core in range(8):
        b = core // 4
        out[b] += res.results[core]["out_part"]
        avg[b] += res.results[core]["avg_part"]
    out += bo
    return out, avg


# revision 9
# speedup vs baseline: 1.1428x; 1.0808x over previous
"""Multi-head causal attention (B=2, T=2048, C=1024, H=16) on 8 TRN2 cores.

Sharding: core c -> batch b = c//4, head-group hg = c%4 (4 heads, 256 dims).
Data parallel on B, tensor parallel on H. Host pre-transposes x and weight
slices; device computes in f32r (full-rate fp32 container) with fp32 PSUM
accumulation; host sums tensor-parallel partials.

Device phases per core:
  1. qT,kT [256,2048] (transposed), v natural [2048,256] (+bias via ones-row MM)
  2. nat-pass per t-tile: scores[t,s] -> +mask -> exp(accum_out=rowsums)
     -> 1/Z columns -> avg_part[t,s] += diag(1/Z_h) @ E_h (PE accumulates heads)
  3. 1/Z rows via PE transpose of the spread 1/Z columns
  4. T-pass per head pair: scoresT[s,t] -> +mask -> exp -> E_T;
     attnT[128,t] += v_pair.T @ E_T (two heads share PSUM via col groups);
     normalize with broadcast 1/Z rows
  5. out_part = attnT_n.T @ Wo.T slice
"""
import math
import sys

sys.path.insert(0, '/opt/trn_rl_repo')

import numpy as np

import concourse.bass as bass
import concourse.tile as tile
from concourse import bacc, mybir
from concourse.bass import ts, ds
from concourse.bass_utils import run_bass_kernel_spmd

F32, F32R = mybir.dt.float32, mybir.dt.float32r
AF = mybir.ActivationFunctionType
ADD, MULT = mybir.AluOpType.add, mybir.AluOpType.mult

B, T, C, H = 2, 2048, 1024, 16
DH = C // H                      # 64
HG = 4                           # heads per core
HD = HG * DH                     # 256
NKT = C // 128                   # 8
NTT = T // 128                   # 16
SCALE = 1.0 / math.sqrt(DH)
NEG = -1.0e30

_CACHE = {}


def _chunks(sb, se):
    """512-aligned chunks covering [sb, se)."""
    out = []
    cb = sb
    while cb < se:
        cw = min(512 * (cb // 512 + 1), se) - cb
        out.append((cb, cw))
        cb += cw
    return out


def _build():
    nc = bacc.Bacc("TRN2", target_bir_lowering=False, debug=False, num_devices=8)
    xt = nc.dram_tensor("xt", [C, T], F32, kind="ExternalInput")
    wqt = nc.dram_tensor("wqt", [C, HD], F32, kind="ExternalInput")
    wkt = nc.dram_tensor("wkt", [C, HD], F32, kind="ExternalInput")
    wvt = nc.dram_tensor("wvt", [C, HD], F32, kind="ExternalInput")
    wot = nc.dram_tensor("wot", [HD, C], F32, kind="ExternalInput")
    bqm = nc.dram_tensor("bqm", [128, 2], F32, kind="ExternalInput")
    bkm = nc.dram_tensor("bkm", [128, 2], F32, kind="ExternalInput")
    bvr = nc.dram_tensor("bvr", [1, HD], F32, kind="ExternalInput")
    mneg_t = nc.dram_tensor("mneg_t", [128, 128], F32, kind="ExternalInput")
    mneg_n = nc.dram_tensor("mneg_n", [128, 128], F32, kind="ExternalInput")
    ident = nc.dram_tensor("ident", [128, 128], F32, kind="ExternalInput")
    out_part = nc.dram_tensor("out_part", [T, C], F32, kind="ExternalOutput")
    avg_part = nc.dram_tensor("avg_part", [T, T], F32, kind="ExternalOutput")

    with tile.TileContext(nc) as tc:
        with tc.tile_pool(name="persist", bufs=1) as pp:
            # ---------- persistent tiles ----------
            qt_s = pp.tile([128, 2, T], F32R, tag="qt")
            kt_s = pp.tile([128, 2, T], F32R, tag="kt")
            vaug = pp.tile([128, NTT, HD], F32R, tag="vaug")
            attnT_n = pp.tile([128, 2, T], F32R, tag="attnT_n")
            wot_s = pp.tile([128, 2, C], F32R, tag="wot")
            nc.gpsimd.dma_start(
                wot_s[:], wot.rearrange("(kt p) n -> p kt n", p=128).bitcast(F32R))
            bq_s = pp.tile([128, 2], F32, tag="bq")
            bk_s = pp.tile([128, 2], F32, tag="bk")
            nc.gpsimd.dma_start(bq_s[:], bqm[:])
            nc.gpsimd.dma_start(bk_s[:], bkm[:])
            bv_row = pp.tile([1, HD], F32R, tag="bv_row")
            nc.gpsimd.dma_start(bv_row[:], bvr[:].bitcast(F32R))
            ones_row = pp.tile([1, 128], F32R, tag="ones_row")
            nc.vector.memset(ones_row[:].bitcast(F32), 1.0)
            mt_s = pp.tile([128, 128], F32, tag="mt_s")
            mn_s = pp.tile([128, 128], F32, tag="mn_s")
            id_s = pp.tile([128, 128], F32, tag="id_s")
            nc.gpsimd.dma_start(mt_s[:], mneg_t[:])
            nc.gpsimd.dma_start(mn_s[:], mneg_n[:])
            nc.gpsimd.dma_start(id_s[:], ident[:])
            # 1/Z: spread columns (per t-tile, head h at col 32h) and row form
            rzcs = pp.tile([128, NTT, 128], F32, tag="rzcs")
            rzr = [pp.tile([1, T], F32, tag=f"rzr{h}", name=f"rzr{h}")
                   for h in range(HG)]

            # ---------- phase 1: projections ----------
            with (
                tc.tile_pool(name="ph1", bufs=1) as p1,
                tc.tile_pool(name="ph1ps", bufs=1, space="PSUM") as p1ps,
            ):
                xt_s = p1.tile([128, NKT, T], F32R, tag="xt")
                for k in range(NKT):
                    nc.gpsimd.dma_start(
                        xt_s[:, k, :], xt[ts(k, 128), :].bitcast(F32R))
                wq_s = p1.tile([128, NKT, HD], F32R, tag="wq")
                wk_s = p1.tile([128, NKT, HD], F32R, tag="wk")
                wv_s = p1.tile([128, NKT, HD], F32R, tag="wv")
                for w_t, dram in [(wq_s, wqt), (wk_s, wkt), (wv_s, wvt)]:
                    nc.gpsimd.dma_start(
                        w_t[:], dram.rearrange("(kt p) n -> p kt n", p=128).bitcast(F32R))

                for dst, w_t, b_t in [(qt_s, wq_s, bq_s), (kt_s, wk_s, bk_s)]:
                    for m in range(2):
                        for tch in range(T // 512):
                            ps = p1ps.tile([128, 512], F32, tag="pj", bufs=2,
                                           name=f"pj{m}_{tch}")
                            for k in range(NKT):
                                nc.tensor.matmul(
                                    ps[:], w_t[:, k, ts(m, 128)],
                                    xt_s[:, k, ts(tch, 512)],
                                    start=(k == 0), stop=(k == NKT - 1))
                            dslice = dst[:, m, ts(tch, 512)]
                            nc.scalar.activation(dslice, ps[:], AF.Copy)
                            nc.vector.tensor_scalar_add(dslice, dslice,
                                                        b_t[:, m:m + 1])
                for j in range(NTT):
                    ps = p1ps.tile([128, HD], F32, tag="pj", bufs=2, name=f"pv{j}")
                    for k in range(NKT):
                        nc.tensor.matmul(ps[:], xt_s[:, k, ts(j, 128)],
                                         wv_s[:, k, :], start=(k == 0), stop=False)
                    nc.tensor.matmul(ps[:], ones_row[:], bv_row[:],
                                     start=False, stop=True)
                    nc.scalar.activation(vaug[:, j, :], ps[:], AF.Copy)

            with (
                tc.tile_pool(name="ph2", bufs=1) as p2,
                tc.tile_pool(name="ph2ps", bufs=1, space="PSUM") as ps2,
            ):
                # ---------- phase 2: nat-pass (avg + rowsums) ----------
                for tt in range(NTT):
                    width = 128 * (tt + 1)
                    nseg = (width + 1023) // 1024
                    racc = p2.tile([128, HG, 2], F32, tag="racc", bufs=2,
                                   name=f"racc{tt}")
                    e_nat = []
                    for h in range(HG):
                        mt, hb = h // 2, 64 * (h % 2)
                        en = p2.tile([128, T], F32R, tag=f"en{h}", name=f"en{tt}_{h}")
                        e_nat.append(en)
                        for si in range(nseg):
                            sb, se = 1024 * si, min(1024 * si + 1024, width)
                            sn = ps2.tile([128, 1024], F32, tag="span", bufs=2,
                                          name=f"sn{tt}_{h}_{si}")
                            for (cb, cw) in _chunks(sb, se):
                                nc.tensor.matmul(
                                    sn[:, ds(cb - sb, cw)],
                                    qt_s[hb:hb + DH, mt, ts(tt, 128)],
                                    kt_s[hb:hb + DH, mt, ds(cb, cw)],
                                    start=True, stop=True)
                            if se == width:  # diagonal block lives here
                                nc.vector.tensor_add(
                                    sn[:, ds(width - 128 - sb, 128)],
                                    sn[:, ds(width - 128 - sb, 128)], mn_s[:])
                            nc.scalar.activation(en[:, ds(sb, se - sb)],
                                                 sn[:, 0:se - sb], AF.Exp,
                                                 scale=SCALE,
                                                 accum_out=racc[:, h, si:si + 1])
                        if nseg < 2:
                            nc.vector.memset(racc[:, h, 1:2], 0.0)
                    zs = p2.tile([128, HG], F32, tag="zs", bufs=2, name=f"zs{tt}")
                    nc.vector.tensor_reduce(zs[:], racc[:],
                                            axis=mybir.AxisListType.X, op=ADD)
                    rz4 = p2.tile([128, HG], F32, tag="rz4", bufs=2, name=f"rz4{tt}")
                    nc.vector.reciprocal(rz4[:], zs[:])
                    nc.vector.tensor_copy(
                        rzcs[:, tt, :].rearrange("p (h q) -> p h q", q=32)[:, :, 0:1],
                        rz4[:].rearrange("p (h q) -> p h q", q=1))
                    # avg accumulation over heads via diagonal matmuls
                    av = ps2.tile([128, T], F32, tag="acc", name=f"av{tt}")
                    for h in range(HG):
                        dg = p2.tile([128, 128], F32R, tag="dg", bufs=2,
                                     name=f"dg{tt}_{h}")
                        nc.vector.tensor_scalar(dg[:], id_s[:],
                                                rzcs[:, tt, 32 * h:32 * h + 1],
                                                None, op0=MULT)
                        for (cb, cw) in _chunks(0, width):
                            nc.tensor.matmul(av[:, ds(cb, cw)], dg[:],
                                             e_nat[h][:, ds(cb, cw)],
                                             start=(h == 0), stop=(h == HG - 1))
                    avs = p2.tile([128, T], F32, tag="avs", bufs=2, name=f"avs{tt}")
                    nc.vector.tensor_scalar_mul(avs[:, 0:width], av[:, 0:width],
                                                1.0 / H)
                    nc.gpsimd.dma_start(avg_part[ts(tt, 128), 0:width],
                                        avs[:, 0:width])

                # ---------- phase 3: 1/Z rows via PE transpose ----------
                for tt in range(NTT):
                    tpz = ps2.tile([128, 128], F32, tag="span", bufs=2,
                                   name=f"tpz{tt}")
                    nc.tensor.transpose(tpz[:], rzcs[:, tt, :], id_s[:])
                    for h in range(HG):
                        nc.vector.tensor_copy(rzr[h][:, ts(tt, 128)],
                                              tpz[32 * h:32 * h + 1, :])

                # ---------- phase 4: T-pass per (head pair, t-half) ----------
                for mp in range(2):
                    for tc2 in range(2):
                        hbase, hend = 1024 * tc2, 1024 * (tc2 + 1)
                        a2 = ps2.tile([64, 2, 1024], F32, tag="acc",
                                      name=f"a2_{mp}_{tc2}")
                        for j in range(8 * (tc2 + 1)):
                            sb = max(128 * j, hbase)
                            for h2 in range(2):
                                hb = 64 * h2
                                sc = ps2.tile([128, 1024], F32, tag="span", bufs=2,
                                              name=f"sc{mp}_{tc2}_{j}_{h2}")
                                for (cb, cw) in _chunks(sb, hend):
                                    nc.tensor.matmul(
                                        sc[:, ds(cb - hbase, cw)],
                                        kt_s[hb:hb + DH, mp, ts(j, 128)],
                                        qt_s[hb:hb + DH, mp, ds(cb, cw)],
                                        start=True, stop=True)
                                if sb == 128 * j:  # diagonal block in this half
                                    nc.vector.tensor_add(
                                        sc[:, ds(sb - hbase, 128)],
                                        sc[:, ds(sb - hbase, 128)], mt_s[:])
                                et = p2.tile([128, 1024], F32R, tag="et", bufs=3,
                                             name=f"et{mp}_{tc2}_{j}_{h2}")
                                nc.scalar.activation(et[:, ds(sb - hbase, hend - sb)],
                                                     sc[:, ds(sb - hbase, hend - sb)],
                                                     AF.Exp, scale=SCALE)
                                for (cb, cw) in _chunks(sb, hend):
                                    nc.tensor.matmul(
                                        a2[:, h2, ds(cb - hbase, cw)],
                                        vaug[:, j, ts(2 * mp + h2, DH)],
                                        et[:, ds(cb - hbase, cw)],
                                        start=(j == 0),
                                        stop=(j % 4 == 3 and cb // 512 == j // 4))
                        for h2 in range(2):
                            hb = 64 * h2
                            rb = p2.tile([DH, 1024], F32, tag="rb", bufs=2,
                                         name=f"rb{mp}_{tc2}_{h2}")
                            nc.gpsimd.partition_broadcast(
                                rb[:], rzr[2 * mp + h2][:, ds(hbase, 1024)])
                            nc.vector.tensor_mul(
                                attnT_n[hb:hb + DH, mp, ds(hbase, 1024)],
                                a2[:, h2, :], rb[:])

                # ---------- phase 5: out-projection ----------
                for tt in range(NTT):
                    po = ps2.tile([128, C], F32,
                                  tag=("span" if tt % 2 == 0 else "acc"),
                                  bufs=(2 if tt % 2 == 0 else 1),
                                  name=f"po{tt}")
                    for nch in range(2):
                        for m in range(2):
                            nc.tensor.matmul(po[:, ts(nch, 512)],
                                             attnT_n[:, m, ts(tt, 128)],
                                             wot_s[:, m, ts(nch, 512)],
                                             start=(m == 0), stop=(m == 1))
                    osb = p2.tile([128, C], F32, tag="osb", bufs=2, name=f"osb{tt}")
                    if tt % 2 == 0:
                        nc.scalar.activation(osb[:], po[:], AF.Copy)
                    else:
                        nc.vector.tensor_copy(osb[:], po[:])
                    nc.gpsimd.dma_start(out_part[ts(tt, 128), :], osb[:])

    nc.compile()
    return nc


def kernel(x, Wq, bq, Wk, bk, Wv, bv, Wo, bo):
    x = np.asarray(x, dtype=np.float32)
    Wq, Wk, Wv, Wo = (np.asarray(a, dtype=np.float32) for a in (Wq, Wk, Wv, Wo))
    bq, bk, bv, bo = (np.asarray(a, dtype=np.float32) for a in (bq, bk, bv, bo))

    if "nc" not in _CACHE:
        _CACHE["nc"] = _build()
    nc = _CACHE["nc"]

    r = np.arange(128)
    mneg = np.where(r[:, None] <= r[None, :], 0.0, NEG).astype(np.float32)
    ident = np.eye(128, dtype=np.float32)

    in_maps = []
    for core in range(8):
        b, hg = core // 4, core % 4
        sl = slice(hg * HD, (hg + 1) * HD)
        in_maps.append({
            "xt": np.ascontiguousarray(x[b].T),
            "wqt": np.ascontiguousarray(Wq[sl, :].T),
            "wkt": np.ascontiguousarray(Wk[sl, :].T),
            "wvt": np.ascontiguousarray(Wv[sl, :].T),
            "wot": np.ascontiguousarray(Wo[:, sl].T),
            "bqm": np.ascontiguousarray(bq[sl].reshape(2, 128).T),
            "bkm": np.ascontiguousarray(bk[sl].reshape(2, 128).T),
            "bvr": np.ascontiguousarray(bv[sl].reshape(1, HD)),
            "mneg_t": mneg,                       # [s,t] keep s<=t
            "mneg_n": np.ascontiguousarray(mneg.T),  # [t,s] keep s<=t
            "ident": ident,
        })

    res = run_bass_kernel_spmd(nc, in_maps, core_ids=list(range(8)),
                               **_CACHE.get("run_kwargs", {}))
    _CACHE["last_result"] = res

    out = np.zeros((B, T, C), dtype=np.float32)
    avg = np.zeros((B, T, T), dtype=np.float32)
    for core in range(8):
        b = core // 4
        out[b] += res.results[core]["out_part"]
        avg[b] += res.results[core]["avg_part"]
    out += bo
    return out, avg
